# revision 1
# baseline (speedup 1.0000x reference)
"""Trainium2 Bass kernel for a 4-branch GCN encoder (con/dep/sem/amr).

Math notes (per branch, per layer):
    reference: x_{l+1} = relu((A_l x W^T + b + x W^T + b) / d_l)
             = relu(((A_l + I) x W^T + 2b) / d_l),   d_l = rowsum(A_l) + 1

We keep the running state un-normalized (division deferred):
    z_0 = D_0 x_0
    z_{l+1} = relu(Abar_l z_l W_l^T + 2b_l),  Abar_l = (A_l + I) D_{l-1 if l>0 else 0}^{-1}
    branch output x_L = z_L / d_{L-1}  (folded into the last ReLU as a
    per-partition activation scale)

On-chip layouts (per example):
    state z:   [T-part, D-free]  -> 4 tiles [128, 256]
    Abar^T:    [j-part, i-free]  -> 4 tiles [128, 512] (PE-transposed from
               natural A tiles; +I and the column scaling folded in)
    U^T = (Abar z)^T accumulates in PSUM as [d-part, i-free] (2 banks), is
    evacuated to SBUF and used as the stationary side of the linear, whose
    output lands back in [T-part, D-free]. No state transposes anywhere.

Matmul dtype mode: "bf16" (fast weight load; ~1e-3 rel err) or "f32r"
(tf32-like, ~2.5e-4 rel err, slower LDWEIGHTS path).

Sharding: data-parallel over batch B=32 across 8 cores (4 examples/core),
weights replicated (pre-transposed on host: W^T with d on partitions).
"""

import os
import sys

import numpy as np

if "/opt/trn_rl_repo" not in sys.path:
    sys.path.insert(0, "/opt/trn_rl_repo")

B, T, D = 32, 512, 256
CON_L, DEP_L, SEM_L, AMR_L = 2, 2, 2, 9
NCORES = 8
BP = B // NCORES  # examples per core
TT = T // 128     # 4 tiles along T
DT = D // 128     # 2 tiles along D

MODE = os.environ.get("GCN_KERNEL_MODE", "bf16")

_PROG_CACHE = {}


def _build_program(mode):
    """Build the single-core Bass/Tile program (same program on all 8 cores)."""
    from contextlib import ExitStack

    import concourse.tile as tile
    from concourse import bacc, mybir

    f32 = mybir.dt.float32
    i32 = mybir.dt.int32
    MD = mybir.dt.bfloat16 if mode == "bf16" else mybir.dt.float32r
    # transpose path dtype: bf16 transposes in bf16 mode; plain f32 otherwise
    TD = mybir.dt.bfloat16 if mode == "bf16" else f32
    RELU = mybir.ActivationFunctionType.Relu
    COPY = mybir.ActivationFunctionType.Copy
    AX = mybir.AxisListType.X

    nc = bacc.Bacc("TRN2", target_bir_lowering=False, debug=False)

    # ---- DRAM I/O (per-core shard shapes) ----
    x0_d = nc.dram_tensor("x0", [BP, T, D], f32, kind="ExternalInput").ap()
    conA_d = nc.dram_tensor("conA", [CON_L, BP, T, T], i32, kind="ExternalInput").ap()
    depA_d = nc.dram_tensor("depA", [BP, T, T], i32, kind="ExternalInput").ap()
    semA_d = nc.dram_tensor("semA", [BP, T, T], f32, kind="ExternalInput").ap()
    amrA_d = nc.dram_tensor("amrA", [BP, T, T], i32, kind="ExternalInput").ap()
    wt_d = {}
    b2_d = {}
    for g, L in (("con", CON_L), ("dep", DEP_L), ("sem", SEM_L), ("amr", AMR_L)):
        # host pre-transposed: wt[l][d][o] = W[l][o][d]; b2[l] = 2*b[l]
        wt_d[g] = nc.dram_tensor(f"wt_{g}", [L, D, D], MD, kind="ExternalInput").ap()
        b2_d[g] = nc.dram_tensor(f"b2_{g}", [L, D], MD, kind="ExternalInput").ap()
    ident_d = nc.dram_tensor("ident", [128, 128], TD, kind="ExternalInput").ap()
    ones_d = nc.dram_tensor("ones_row", [1, T], MD, kind="ExternalInput").ap()

    out_d = {}
    for g in ("con", "dep", "sem", "amr"):
        out_d[g] = nc.dram_tensor(f"{g}_out", [BP, T, D], f32, kind="ExternalOutput").ap()

    big = mode == "bf16"  # bf16 tiles are half size; deepen pipelines
    with tile.TileContext(nc) as tc, ExitStack() as ctx:
        const_pool = ctx.enter_context(tc.tile_pool(name="const", bufs=1))
        wt_pool = ctx.enter_context(tc.tile_pool(name="wt", bufs=1))
        x0_pool = ctx.enter_context(tc.tile_pool(name="x0", bufs=12 if big else 6))
        xb_pool = ctx.enter_context(tc.tile_pool(name="xb", bufs=4 if big else 2))
        z_pool = ctx.enter_context(tc.tile_pool(name="z", bufs=6 if big else 4))
        an_pool = ctx.enter_context(tc.tile_pool(name="an", bufs=4))
        at_pool = ctx.enter_context(tc.tile_pool(name="at", bufs=4))
        u_pool = ctx.enter_context(tc.tile_pool(name="usb", bufs=6 if big else 4))
        rs_pool = ctx.enter_context(tc.tile_pool(name="rs", bufs=12))
        u_psum = ctx.enter_context(tc.tile_pool(name="u_ps", bufs=3, space="PSUM"))
        y_psum = ctx.enter_context(tc.tile_pool(name="y_ps", bufs=3, space="PSUM"))
        tp_psum = ctx.enter_context(tc.tile_pool(name="tp_ps", bufs=2, space="PSUM"))

        # ---- constants ----
        ident_sb = const_pool.tile([128, 128], TD, name="ident_sb")
        nc.sync.dma_start(ident_sb[:], ident_d[:])
        ones_sb = const_pool.tile([1, T], MD, name="ones_sb")
        nc.sync.dma_start(ones_sb[:], ones_d[:])

        wt_sb = {}
        b2_sb = {}
        for g, L in (("con", CON_L), ("dep", DEP_L), ("sem", SEM_L), ("amr", AMR_L)):
            b2t = const_pool.tile([1, L * D], MD, name=f"b2_{g}_sb")
            nc.sync.dma_start(b2t[:], b2_d[g].rearrange("l o -> (l o)")[None, :])
            b2_sb[g] = b2t
            tiles = []
            for l in range(L):
                w = wt_pool.tile([128, DT * D], MD, name=f"wt_{g}{l}_sb")
                # w[p, dt*D + o] = W^T[dt*128 + p, o]
                nc.sync.dma_start(
                    w[:].rearrange("p (dt o) -> p dt o", o=D),
                    wt_d[g][l].rearrange("(dt p) o -> p dt o", p=128),
                )
                tiles.append(w)
            wt_sb[g] = tiles

        def gcn_branch(e, tag, L, adj_for_layer, x0_tiles):
            """adj_for_layer(l) -> (dram AP [T,T], needs_cast) or None if same as l-1."""
            wt = wt_sb[tag]
            b2 = b2_sb[tag]
            aT = None
            i4_prev = None
            i4_last = None
            z = None
            for l in range(L):
                adj = adj_for_layer(l)
                if adj is not None:
                    src, cast = adj
                    an = []
                    for it in range(TT):
                        t = an_pool.tile([128, T], TD, name=f"an_{tag}{e}{l}{it}",
                                         tag=f"an_{tag}", bufs=8 if big else 4)
                        if cast or TD != f32:
                            nc.gpsimd.dma_start(t[:], src[it * 128:(it + 1) * 128, :])
                        else:
                            nc.sync.dma_start(t[:], src[it * 128:(it + 1) * 128, :])
                        an.append(t)
                    # A' = A + I in SBUF (diagonal blocks), exact in bf16
                    for it in range(TT):
                        nc.vector.tensor_add(
                            an[it][:, it * 128:(it + 1) * 128],
                            an[it][:, it * 128:(it + 1) * 128],
                            ident_sb[:],
                        )
                    # d = rowsum(A') ; inv = 1/d
                    d4 = rs_pool.tile([128, TT], f32, name=f"d4_{tag}{e}{l}", tag="d4")
                    for it in range(TT):
                        nc.vector.reduce_sum(d4[:, it:it + 1], an[it][:], axis=AX)
                    i4 = rs_pool.tile([128, TT], f32, name=f"i4_{tag}{e}{l}", tag="i4")
                    nc.vector.reciprocal(i4[:], d4[:])
                    scale_i4 = i4 if l == 0 else i4_prev
                    # transpose A' -> Abar^T tiles (cols scaled by prev inv)
                    aT = []
                    for jt in range(TT):
                        tp = tp_psum.tile([128, T], TD, name=f"tp_{tag}{e}{l}{jt}", tag="tp")
                        for it in range(TT):
                            nc.tensor.matmul(
                                tp[:, it * 128:(it + 1) * 128],
                                an[it][:, jt * 128:(jt + 1) * 128],
                                ident_sb[:],
                                is_transpose=True,
                                start=(it == 0),
                                stop=(it == TT - 1),
                            )
                        a_t = at_pool.tile([128, T], MD, name=f"aT_{tag}{e}{l}{jt}",
                                           tag=f"at_{tag}", bufs=(12 if tag == "amr" else 8) if big else (8 if tag in ("con", "amr") else 4))
                        if jt % 2 == 0:
                            nc.scalar.activation(a_t[:], tp[:], COPY, scale=scale_i4[:, jt:jt + 1])
                        else:
                            nc.vector.tensor_scalar_mul(a_t[:], tp[:], scale_i4[:, jt:jt + 1])
                        aT.append(a_t)
                    i4_prev = i4
                    i4_last = i4
                    if l == 0:
                        # z_0 = D_0 * x_0 (stored as t-pair tiles [128, 2*D])
                        z = []
                        for jp in range(TT // 2):
                            xb = xb_pool.tile([128, 2 * D], MD, name=f"xb_{tag}{e}{jp}", tag=f"xb_{tag}")
                            for ts_ in range(2):
                                t_i = 2 * jp + ts_
                                nc.vector.tensor_scalar_mul(
                                    xb[:, ts_ * D:(ts_ + 1) * D],
                                    x0_tiles[t_i][:],
                                    d4[:, t_i:t_i + 1],
                                )
                            z.append(xb)

                def z_slice(jt, dt):
                    # lhsT block [128, 128] for t-block jt, d-block dt
                    return z[jt // 2][:, (jt % 2) * D + dt * 128:(jt % 2) * D + (dt + 1) * 128]

                # U^T = (Abar z)^T : accumulate [d-part, i-free]
                u_sb = []
                for dt in range(DT):
                    up = u_psum.tile([128, T], f32, name=f"ups_{tag}{e}{l}{dt}", tag="u")
                    for jt in range(TT):
                        nc.tensor.matmul(
                            up[:],
                            z_slice(jt, dt),
                            aT[jt][:],
                            start=(jt == 0),
                            stop=(jt == TT - 1),
                        )
                    ut = u_pool.tile([128, T], MD, name=f"usb_{tag}{e}{l}{dt}", tag="usb")
                    if dt == 0:
                        nc.vector.tensor_copy(ut[:], up[:])
                    else:
                        nc.scalar.copy(ut[:], up[:])
                    u_sb.append(ut)
                # z_{l+1} = relu(U W^T + 2b)   [T-part, D-free], two t-blocks per PSUM bank
                z_next = []
                for jp in range(TT // 2):
                    yp = y_psum.tile([128, 2 * D], f32, name=f"yps_{tag}{e}{l}{jp}", tag="y")
                    for ts_ in range(2):
                        t_i = 2 * jp + ts_
                        nc.tensor.matmul(
                            yp[:, ts_ * D:(ts_ + 1) * D],
                            ones_sb[0:1, t_i * 128:(t_i + 1) * 128],
                            b2[0:1, l * D:(l + 1) * D],
                            start=(ts_ == 0),
                            stop=False,
                        )
                    for dt in range(DT):
                        for ts_ in range(2):
                            t_i = 2 * jp + ts_
                            nc.tensor.matmul(
                                yp[:, ts_ * D:(ts_ + 1) * D],
                                u_sb[dt][:, t_i * 128:(t_i + 1) * 128],
                                wt[l][:, dt * D:(dt + 1) * D],
                                start=False,
                                stop=(ts_ == 1 and dt == DT - 1),
                            )
                    if l == L - 1:
                        # final: x_L = relu(y) / d_{L-1}; per-partition scale differs
                        # per t-block, so two separate scaled ReLUs
                        for ts_ in range(2):
                            t_i = 2 * jp + ts_
                            zt = z_pool.tile([128, D], f32, name=f"z_{tag}{e}{l}{t_i}", tag=f"zf_{tag}")
                            nc.scalar.activation(zt[:], yp[:, ts_ * D:(ts_ + 1) * D], RELU,
                                                 scale=i4_last[:, t_i:t_i + 1])
                            nc.sync.dma_start(out_d[tag][e][t_i * 128:(t_i + 1) * 128, :], zt[:])
                    else:
                        zt = z_pool.tile([128, 2 * D], MD, name=f"z_{tag}{e}{l}{jp}", tag=f"z_{tag}")
                        nc.scalar.activation(zt[:], yp[:], RELU)
                        z_next.append(zt)
                z = z_next

        for e in range(BP):
            x0_tiles = []
            for t_i in range(TT):
                xt = x0_pool.tile([128, D], f32, name=f"x0_{e}{t_i}", tag="x0")
                nc.sync.dma_start(xt[:], x0_d[e][t_i * 128:(t_i + 1) * 128, :])
                x0_tiles.append(xt)
            gcn_branch(e, "amr", AMR_L, lambda l, e=e: (amrA_d[e], True) if l == 0 else None, x0_tiles)
            gcn_branch(e, "con", CON_L, lambda l, e=e: (conA_d[l][e], True), x0_tiles)
            gcn_branch(e, "dep", DEP_L, lambda l, e=e: (depA_d[e], True) if l == 0 else None, x0_tiles)
            gcn_branch(e, "sem", SEM_L, lambda l, e=e: (semA_d[e], False) if l == 0 else None, x0_tiles)

    nc.compile()
    return nc


def _get_program(mode=MODE):
    if mode not in _PROG_CACHE:
        _PROG_CACHE[mode] = _build_program(mode)
    return _PROG_CACHE[mode]


def _make_in_maps(inputs, mode=MODE):
    import ml_dtypes

    wdt = np.float32 if mode != "bf16" else ml_dtypes.bfloat16

    x = np.ascontiguousarray(inputs["inputs"], dtype=np.float32)
    con = np.ascontiguousarray(inputs["con_adj"], dtype=np.int32)
    dep = np.ascontiguousarray(inputs["dep_adj"], dtype=np.int32)
    sem = np.ascontiguousarray(inputs["seman_adj"], dtype=np.float32)
    amr = np.ascontiguousarray(inputs["amr_adj"], dtype=np.int32)

    tdt = np.float32 if mode != "bf16" else ml_dtypes.bfloat16
    const = {
        "ident": np.eye(128, dtype=tdt),
        "ones_row": np.ones((1, T), dtype=wdt),
    }
    for g in ("con", "dep", "sem", "amr"):
        W = np.asarray(inputs[f"W_{g}"], dtype=np.float32)
        b = np.asarray(inputs[f"b_{g}"], dtype=np.float32)
        const[f"wt_{g}"] = np.ascontiguousarray(np.transpose(W, (0, 2, 1))).astype(wdt)
        const[f"b2_{g}"] = np.ascontiguousarray(2.0 * b).astype(wdt)

    in_maps = []
    for c in range(NCORES):
        s = slice(c * BP, (c + 1) * BP)
        m = dict(const)
        m["x0"] = x[s]
        m["conA"] = np.ascontiguousarray(con[:, s])
        m["depA"] = dep[s]
        m["semA"] = sem[s]
        m["amrA"] = amr[s]
        in_maps.append(m)
    return in_maps


def kernel(trace=False, **inputs):
    from concourse.bass_utils import run_bass_kernel_spmd

    nc = _get_program()
    in_maps = _make_in_maps(inputs)
    res = run_bass_kernel_spmd(nc, in_maps, core_ids=list(range(NCORES)), trace=trace)
    outs = []
    for g in ("con", "dep", "sem", "amr"):
        full = np.concatenate([res.results[c][f"{g}_out"] for c in range(NCORES)], axis=0)
        outs.append(full.astype(np.float32))
    if trace:
        kernel.last_exec_time_ns = res.exec_time_ns
        kernel.last_results = res
    return tuple(outs)

